# revision 15
# baseline (speedup 1.0000x reference)
"""Trainium2 Bass kernel for GQA attention (B=1, S=2048, D=2048, H=32, KV=8, HD=64).

Tensor-parallel over heads across 8 NeuronCores: core i holds q-heads
[4i, 4i+4) and kv-head i; each core computes its partial o_proj output and the
host sums the 8 partials (Megatron all-reduce done host-side).

Self-contained: only imports concourse (on sys.path in the container).
"""

import os
import sys

import ml_dtypes
import numpy as np

if "/opt/trn_rl_repo" not in sys.path and not any(
    p.endswith("trn_rl_repo") for p in sys.path
):
    sys.path.insert(0, "/opt/trn_rl_repo")

import concourse.bass as bass
import concourse.mybir as mybir
import concourse.tile as tile
from concourse import bacc
from concourse.bass_utils import run_bass_kernel_spmd
from concourse.masks import make_identity

F32 = mybir.dt.float32
F32R = mybir.dt.float32r
BF16 = mybir.dt.bfloat16


def _r(ap):
    return ap.bitcast(F32R)
AF = mybir.ActivationFunctionType
ALU = mybir.AluOpType

S = 2048
D = 2048
H = 32
KV = 8
HD = 64
NCORES = 8
HQ = H // NCORES  # 4 q heads per core
NKC = S // 128  # 16 key chunks
NQB = 4  # q blocks of 512
QBW = 512
NSB = 4  # s blocks of 512 in projection
SBW = 512
DCH = D // 128  # 16 contraction chunks

def _build_nc():
    nc = bacc.Bacc("TRN2", target_bir_lowering=False, debug=False, num_devices=NCORES)

    xt_d = nc.declare_dram_parameter("xt", [D, S], BF16, isOutput=False)
    wqkv_d = nc.declare_dram_parameter("wqkv", [D, 384], BF16, isOutput=False)
    wo_d = nc.declare_dram_parameter("wo", [2, 128, D], BF16, isOutput=False)
    cos_d = nc.declare_dram_parameter("cos", [128, S], F32, isOutput=False)
    sin_d = nc.declare_dram_parameter("sin", [128, S], F32, isOutput=False)
    sel_d = nc.declare_dram_parameter("sel", [16, 16 * 128], BF16, isOutput=False)
    y_d = nc.declare_dram_parameter("y", [S, D], F32, isOutput=True)

    with tile.TileContext(nc) as tc:
        with (
            tc.tile_pool(name="glob", bufs=1) as glob,
            tc.tile_pool(name="globp", bufs=2, space="PSUM") as globp,
        ):
            ktdup = glob.tile([128, S], BF16, tag="ktdup")
            v_s = glob.tile([128, NKC, 65], BF16, tag="v_s")
            outA = glob.tile([128, S], BF16, tag="outA")
            outB = glob.tile([128, S], BF16, tag="outB")
            ao = glob.tile([128, 2, S], BF16, tag="ao")
            sel_s = glob.tile([16, 16 * 128], BF16, tag="sel_s")
            ident = glob.tile([128, 128], F32, tag="ident")
            wo_s = glob.tile([128, 2, D], BF16, tag="wo_s")
            sums_all = glob.tile([16, QBW], F32, tag="sums_all")
            rcp_all = glob.tile([16, QBW], BF16, tag="rcp_all")
            rcp_f32 = glob.tile([16, QBW], F32, tag="rcp_f32")
            rcp_scr = glob.tile([16, QBW], F32, tag="rcp_scr")

            make_identity(nc, ident[:])
            nc.sync.dma_start(sel_s[:], sel_d[:])
            for ch in range(2):
                nc.sync.dma_start(wo_s[:, ch, :], wo_d[ch])
            nc.vector.memset(v_s[:, :, 64], 1.0)

            # ---------------- P1: projections + RoPE + v transpose -------------
            with (
                tc.tile_pool(name="p1", bufs=1) as p1,
                tc.tile_pool(name="xp", bufs=2) as xp,
                tc.tile_pool(name="tmpp", bufs=2) as tmpp,
                tc.tile_pool(name="ps1", bufs=6, space="PSUM") as ps1,
                tc.tile_pool(name="pstr", bufs=2, space="PSUM") as pstr,
            ):
                wq_s = p1.tile([128, DCH, 384], BF16, tag="wq_s")
                nc.sync.dma_start(
                    wq_s[:], wqkv_d.rearrange("(ko p) n -> p ko n", p=128)
                )
                cos_s = p1.tile([128, S], F32, tag="cos_s")
                sin_s = p1.tile([128, S], F32, tag="sin_s")
                nc.sync.dma_start(cos_s[:], cos_d[:])
                nc.sync.dma_start(sin_s[:], sin_d[:])
                kvraw = p1.tile([128, S], F32, tag="kvraw")
                kswap = p1.tile([64, S], F32, tag="kswap")

                xt_r = xt_d.rearrange("(ko p) s -> p ko s", p=128)
                for sb in range(NSB):
                    sbc = slice(sb * SBW, (sb + 1) * SBW)
                    xblk = xp.tile([128, DCH, SBW], BF16, tag="xblk")
                    nc.sync.dma_start(xblk[:], xt_r[:, :, sbc])
                    psA = ps1.tile([128, SBW], F32, tag="proj")
                    psB = ps1.tile([128, SBW], F32, tag="proj")
                    psKV = ps1.tile([128, SBW], F32, tag="proj")
                    for ps_t, col0 in ((psA, 0), (psB, 128), (psKV, 256)):
                        for kc in range(DCH):
                            nc.tensor.matmul(
                                ps_t[:],
                                lhsT=wq_s[:, kc, col0 : col0 + 128],
                                rhs=xblk[:, kc, :],
                                start=(kc == 0),
                                stop=(kc == DCH - 1),
                            )
                    # RoPE on the 4 q heads (A = first-half dims, B = second)
                    tmp = tmpp.tile([128, SBW], F32, tag="tmp")
                    nc.vector.tensor_tensor(
                        outA[:, sbc], psA[:], cos_s[:, sbc], ALU.mult
                    )
                    nc.vector.tensor_tensor(tmp[:], psB[:], sin_s[:, sbc], ALU.mult)
                    nc.vector.tensor_tensor(
                        outA[:, sbc], outA[:, sbc], tmp[:], ALU.subtract
                    )
                    tmp2 = tmpp.tile([128, SBW], F32, tag="tmp")
                    nc.vector.tensor_tensor(
                        outB[:, sbc], psB[:], cos_s[:, sbc], ALU.mult
                    )
                    nc.vector.tensor_tensor(tmp2[:], psA[:], sin_s[:, sbc], ALU.mult)
                    nc.vector.tensor_tensor(
                        outB[:, sbc], outB[:, sbc], tmp2[:], ALU.add
                    )
                    # evict k|v rows
                    nc.scalar.activation(kvraw[:, sbc], psKV[:], AF.Copy)

                    # k RoPE on this s-block: kswap = [k_hi; k_lo]
                    nc.sync.dma_start(kswap[0:32, sbc], kvraw[32:64, sbc])
                    nc.sync.dma_start(kswap[32:64, sbc], kvraw[0:32, sbc])
                    nc.vector.tensor_tensor(
                        ktdup[0:64, sbc], kvraw[0:64, sbc], cos_s[0:64, sbc], ALU.mult
                    )
                    tmpk = tmpp.tile([64, SBW], F32, tag="tmpk")
                    nc.vector.tensor_tensor(
                        tmpk[:], kswap[:, sbc], sin_s[0:64, sbc], ALU.mult
                    )
                    nc.vector.tensor_tensor(
                        ktdup[0:32, sbc], ktdup[0:32, sbc], tmpk[0:32, :],
                        ALU.subtract,
                    )
                    nc.vector.tensor_tensor(
                        ktdup[32:64, sbc], ktdup[32:64, sbc], tmpk[32:64, :],
                        ALU.add,
                    )
                    nc.sync.dma_start(ktdup[64:128, sbc], ktdup[0:64, sbc])

                    # v: [64, 512] -> 4 key-chunk tiles [128, 64] via PE transpose
                    for c in range(4 * sb, 4 * sb + 4):
                        ptr = pstr.tile([128, 64], F32, tag="ptr")
                        nc.tensor.transpose(
                            ptr[:],
                            kvraw[64:128, c * 128 : (c + 1) * 128],
                            ident[64:128, 64:128],
                        )
                        nc.vector.tensor_copy(v_s[:, c, 0:64], ptr[:])

            # ---------------- P2: attention ------------------------------------
            with (
                tc.tile_pool(name="qsp", bufs=2) as qsp,
                tc.tile_pool(name="ptp", bufs=6) as ptp,
                tc.tile_pool(name="stgp", bufs=3) as stgp,
                tc.tile_pool(name="pssc", bufs=2, space="PSUM") as pssc,
                tc.tile_pool(name="pso_p", bufs=1, space="PSUM") as pso_p,
                tc.tile_pool(name="psbc", bufs=2, space="PSUM") as psbc,
            ):
                for h in range(HQ):
                    hc = slice(32 * h, 32 * h + 32)
                    qs = qsp.tile([128, S], BF16, tag="qs")
                    for sb in range(NSB):
                        sbc = slice(sb * SBW, (sb + 1) * SBW)
                        nc.sync.dma_start(qs[0:32, sbc], outA[hc, sbc])
                        nc.sync.dma_start(qs[32:64, sbc], outB[hc, sbc])
                        nc.sync.dma_start(qs[64:96, sbc], outA[hc, sbc])
                        nc.sync.dma_start(qs[96:128, sbc], outB[hc, sbc])
                    for qb in range(NQB):
                        q0 = qb * QBW
                        nkc = 4 * (qb + 1)
                        pso = pso_p.tile([128, 1024], F32, tag="pso")
                        for pair in range(nkc // 2):
                            cA, cB = 2 * pair, 2 * pair + 1
                            psc = pssc.tile([128, 1024], F32, tag="psc")
                            ptt = ptp.tile([128, 1024], BF16, tag="ptt")
                            for c, half, r0 in ((cA, 0, 0), (cB, 1, 64)):
                                kc0 = c * 128
                                d = max(0, kc0 - q0)
                                nc.tensor.matmul(
                                    psc[:, half * 512 + d : half * 512 + 512],
                                    lhsT=ktdup[r0 : r0 + 64, kc0 : kc0 + 128],
                                    rhs=qs[r0 : r0 + 64, q0 + d : q0 + QBW],
                                    start=True,
                                    stop=True,
                                    tile_position=(r0, 0),
                                )
                            dA = max(0, cA * 128 - q0)
                            dB = max(0, cB * 128 - q0)
                            if dA == 0 and dB == 0:
                                nc.scalar.activation(ptt[:], psc[:], AF.Exp)
                            else:
                                nc.scalar.activation(
                                    ptt[:, dA:512], psc[:, dA:512], AF.Exp
                                )
                                nc.scalar.activation(
                                    ptt[:, 512 + dB : 1024],
                                    psc[:, 512 + dB : 1024],
                                    AF.Exp,
                                )
                            for c, half in ((cA, 0), (cB, 1)):
                                kc0 = c * 128
                                if kc0 + 127 > q0 + 0:
                                    ww = min(512, (kc0 - q0) + 128)
                                    sl = slice(half * 512, half * 512 + ww)
                                    nc.gpsimd.affine_select(
                                        out=ptt[:, sl],
                                        in_=ptt[:, sl],
                                        compare_op=ALU.is_ge,
                                        fill=0.0,
                                        base=q0 - kc0,
                                        channel_multiplier=-1,
                                        pattern=[[1, ww]],
                                    )
                            for c, half in ((cA, 0), (cB, 1)):
                                for r0, b0 in ((0, 0), (64, 512)):
                                    nc.tensor.matmul(
                                        pso[0:65, b0 : b0 + 512],
                                        lhsT=v_s[r0 : r0 + 64, c, :],
                                        rhs=ptt[
                                            r0 : r0 + 64,
                                            half * 512 : half * 512 + 512,
                                        ],
                                        start=(c == 0),
                                        stop=(c == nkc - 1),
                                        tile_position=(r0, 0),
                                    )
                        # evict raw attn out + sums
                        idx = 4 * h + qb
                        ch = h // 2
                        rr = 64 * (h % 2)
                        stg = stgp.tile([128, QBW], BF16, tag="stg")
                        tmpb = stgp.tile([128, QBW], F32, tag="tmpb")
                        nc.vector.tensor_copy(tmpb[0:65, :], pso[0:65, 512:1024])
                        nc.vector.tensor_tensor(
                            stg[0:65, :], pso[0:65, 0:512], tmpb[0:65, :], ALU.add
                        )
                        nc.sync.dma_start(
                            ao[rr : rr + 64, ch, q0 : q0 + QBW], stg[0:64, :]
                        )
                        nc.gpsimd.dma_start(
                            sums_all[idx : idx + 1, :], stg[64:65, :]
                        )

                    # normalize this head: recip of its 4 sums rows, then
                    # broadcast via select-matmul and scale ao in place
                    nc.vector.reciprocal_approx_accurate(
                        rcp_f32[:], sums_all[:], rcp_scr[:]
                    )
                    nc.vector.tensor_copy(rcp_all[:], rcp_f32[:])
                    for qb in range(NQB):
                        idx = 4 * h + qb
                        ch = h // 2
                        rr = 64 * (h % 2)
                        q0 = qb * QBW
                        pbc = psbc.tile([128, QBW], F32, tag="pbc")
                        nc.tensor.matmul(
                            pbc[:],
                            lhsT=sel_s[:, idx * 128 : (idx + 1) * 128],
                            rhs=rcp_all[:],
                            start=True,
                            stop=True,
                        )
                        nc.vector.tensor_tensor(
                            ao[rr : rr + 64, ch, q0 : q0 + QBW],
                            ao[rr : rr + 64, ch, q0 : q0 + QBW],
                            pbc[rr : rr + 64, :],
                            ALU.mult,
                        )


            # ---------------- P3: o_proj ---------------------------------------
            with (
                tc.tile_pool(name="yp", bufs=4) as yp,
                tc.tile_pool(name="psy", bufs=4, space="PSUM") as psy_p,
            ):
                for st in range(16):
                    for ob in range(4):
                        psy = psy_p.tile([128, 512], F32, tag="psy")
                        for ch in range(2):
                            nc.tensor.matmul(
                                psy[:],
                                lhsT=ao[:, ch, st * 128 : (st + 1) * 128],
                                rhs=wo_s[:, ch, ob * 512 : (ob + 1) * 512],
                                start=(ch == 0),
                                stop=(ch == 1),
                            )
                        ysb = yp.tile([128, 512], F32, tag="ysb")
                        if (st * 4 + ob) % 2 == 0:
                            nc.scalar.activation(ysb[:], psy[:], AF.Copy)
                        else:
                            nc.vector.tensor_copy(ysb[:], psy[:])
                        nc.gpsimd.dma_start(
                            y_d[st * 128 : (st + 1) * 128, ob * 512 : (ob + 1) * 512],
                            ysb[:],
                        )
    nc.compile()
    return nc


def _prep_inputs(x, Wq, Wk, Wv, Wo, inv_freq):
    """Host-side sharding + layout prep. Returns in_maps for the 8 cores."""
    x = np.ascontiguousarray(np.asarray(x, dtype=np.float32).reshape(S, D))
    xt = np.ascontiguousarray(x.T)  # [D, S]

    pos = np.arange(S, dtype=np.float64)
    inv = np.asarray(inv_freq, dtype=np.float64)  # [32]
    freqs = pos[None, :] * inv[:, None]  # [32, S]
    cos32 = np.cos(freqs).astype(np.float32)
    sin32 = np.sin(freqs).astype(np.float32)
    cos_tab = np.tile(cos32, (4, 1))  # [128, S]
    sin_tab = np.tile(sin32, (4, 1))
    sel = np.zeros((16, 16 * 128), dtype=np.float32)
    for i in range(16):
        h = i // 4
        rr = 64 * (h % 2)
        sel[i, i * 128 + rr : i * 128 + rr + 64] = 1.0

    in_maps = []
    for i in range(NCORES):
        wq_l = Wq[256 * i : 256 * (i + 1)].astype(np.float32) * 0.125  # [256, D]
        wk_l = Wk[64 * i : 64 * (i + 1)].astype(np.float32)  # [64, D]
        wv_l = Wv[64 * i : 64 * (i + 1)].astype(np.float32)  # [64, D]
        # A-tile: first-half dims of the 4 heads; B-tile: second halves
        wA = np.concatenate(
            [wq_l[64 * h : 64 * h + 32] for h in range(HQ)], axis=0
        )  # [128, D]
        wB = np.concatenate(
            [wq_l[64 * h + 32 : 64 * h + 64] for h in range(HQ)], axis=0
        )
        wkv = np.concatenate([wk_l, wv_l], axis=0)  # [128, D]
        wqkv = np.ascontiguousarray(
            np.concatenate([wA, wB, wkv], axis=0).T
        )  # [D, 384]
        wo_l = Wo[:, 256 * i : 256 * (i + 1)].astype(np.float32)  # [D, 256]
        wo_t = np.ascontiguousarray(wo_l.T.reshape(2, 128, D))  # [2, 128, D]
        in_maps.append(
            {
                "xt": xt.astype(ml_dtypes.bfloat16),
                "wqkv": wqkv.astype(ml_dtypes.bfloat16),
                "wo": wo_t.astype(ml_dtypes.bfloat16),
                "cos": cos_tab,
                "sin": sin_tab,
                "sel": sel.astype(ml_dtypes.bfloat16),
            }
        )
    return in_maps


_NC_CACHE = None


def kernel(x, Wq, Wk, Wv, Wo, inv_freq):
    global _NC_CACHE
    if _NC_CACHE is None:
        _NC_CACHE = _build_nc()
    nc = _NC_CACHE
    in_maps = _prep_inputs(x, Wq, Wk, Wv, Wo, inv_freq)
    trace = bool(int(os.environ.get("BASS_KERNEL_TRACE", "0")))
    res = run_bass_kernel_spmd(nc, in_maps, list(range(NCORES)), trace=trace)
    if trace:
        kernel.last_results = res
    y = np.zeros((S, D), dtype=np.float32)
    for i in range(NCORES):
        y += res.results[i]["y"]
    return y.reshape(1, S, D)


# revision 16
# speedup vs baseline: 1.0998x; 1.0998x over previous
"""Trainium2 Bass kernel for GQA attention (B=1, S=2048, D=2048, H=32, KV=8, HD=64).

Tensor-parallel over heads across 8 NeuronCores: core i holds q-heads
[4i, 4i+4) and kv-head i; each core computes its partial o_proj output and the
host sums the 8 partials (Megatron all-reduce done host-side).

Self-contained: only imports concourse (on sys.path in the container).
"""

import os
import sys

import ml_dtypes
import numpy as np

if "/opt/trn_rl_repo" not in sys.path and not any(
    p.endswith("trn_rl_repo") for p in sys.path
):
    sys.path.insert(0, "/opt/trn_rl_repo")

import concourse.bass as bass
import concourse.mybir as mybir
import concourse.tile as tile
from concourse import bacc
from concourse.bass_utils import run_bass_kernel_spmd
from concourse.masks import make_identity

F32 = mybir.dt.float32
F32R = mybir.dt.float32r
BF16 = mybir.dt.bfloat16


def _r(ap):
    return ap.bitcast(F32R)
AF = mybir.ActivationFunctionType
ALU = mybir.AluOpType

S = 2048
D = 2048
H = 32
KV = 8
HD = 64
NCORES = 8
HQ = H // NCORES  # 4 q heads per core
NKC = S // 128  # 16 key chunks
NQB = 4  # q blocks of 512
QBW = 512
NSB = 4  # s blocks of 512 in projection
SBW = 512
DCH = D // 128  # 16 contraction chunks

def _build_nc():
    nc = bacc.Bacc("TRN2", target_bir_lowering=False, debug=False, num_devices=NCORES)

    xt_d = nc.declare_dram_parameter("xt", [D, S], BF16, isOutput=False)
    wqkv_d = nc.declare_dram_parameter("wqkv", [D, 384], BF16, isOutput=False)
    wo_d = nc.declare_dram_parameter("wo", [2, 128, D], BF16, isOutput=False)
    cos_d = nc.declare_dram_parameter("cos", [128, S], F32, isOutput=False)
    sin_d = nc.declare_dram_parameter("sin", [128, S], F32, isOutput=False)
    sel_d = nc.declare_dram_parameter("sel", [16, 16 * 128], BF16, isOutput=False)
    y_d = nc.declare_dram_parameter("y", [S, D], F32, isOutput=True)

    with tile.TileContext(nc) as tc:
        with (
            tc.tile_pool(name="glob", bufs=1) as glob,
            tc.tile_pool(name="globp", bufs=2, space="PSUM") as globp,
        ):
            ktdup = glob.tile([128, S], BF16, tag="ktdup")
            v_s = glob.tile([128, NKC, 65], BF16, tag="v_s")
            outA = glob.tile([128, S], BF16, tag="outA")
            outB = glob.tile([128, S], BF16, tag="outB")
            ao = glob.tile([128, 2, S], BF16, tag="ao")
            sel_s = glob.tile([16, 16 * 128], BF16, tag="sel_s")
            ident = glob.tile([128, 128], F32, tag="ident")
            wo_s = glob.tile([128, 2, D], BF16, tag="wo_s")
            sums_all = glob.tile([16, QBW], F32, tag="sums_all")
            rcp_all = glob.tile([16, QBW], BF16, tag="rcp_all")
            rcp_f32 = glob.tile([16, QBW], F32, tag="rcp_f32")
            rcp_scr = glob.tile([16, QBW], F32, tag="rcp_scr")

            make_identity(nc, ident[:])
            nc.sync.dma_start(sel_s[:], sel_d[:])
            for ch in range(2):
                nc.sync.dma_start(wo_s[:, ch, :], wo_d[ch])
            nc.vector.memset(v_s[:, :, 64], 1.0)

            # ---------------- P1: projections + RoPE + v transpose -------------
            with (
                tc.tile_pool(name="p1", bufs=1) as p1,
                tc.tile_pool(name="xp", bufs=3) as xp,
                tc.tile_pool(name="tmpp", bufs=4) as tmpp,
                tc.tile_pool(name="ps1", bufs=6, space="PSUM") as ps1,
                tc.tile_pool(name="pstr", bufs=2, space="PSUM") as pstr,
            ):
                wq_s = p1.tile([128, DCH, 384], BF16, tag="wq_s")
                nc.sync.dma_start(
                    wq_s[:], wqkv_d.rearrange("(ko p) n -> p ko n", p=128)
                )
                cos_s = p1.tile([128, S], F32, tag="cos_s")
                sin_s = p1.tile([128, S], F32, tag="sin_s")
                nc.sync.dma_start(cos_s[:], cos_d[:])
                nc.sync.dma_start(sin_s[:], sin_d[:])
                kvraw = p1.tile([128, S], F32, tag="kvraw")
                kswap = p1.tile([64, S], F32, tag="kswap")

                xt_r = xt_d.rearrange("(ko p) s -> p ko s", p=128)
                for sb in range(NSB):
                    sbc = slice(sb * SBW, (sb + 1) * SBW)
                    xblk = xp.tile([128, DCH, SBW], BF16, tag="xblk")
                    nc.sync.dma_start(xblk[:], xt_r[:, :, sbc])
                    psA = ps1.tile([128, SBW], F32, tag="proj")
                    psB = ps1.tile([128, SBW], F32, tag="proj")
                    psKV = ps1.tile([128, SBW], F32, tag="proj")
                    for ps_t, col0 in ((psA, 0), (psB, 128), (psKV, 256)):
                        for kc in range(DCH):
                            nc.tensor.matmul(
                                ps_t[:],
                                lhsT=wq_s[:, kc, col0 : col0 + 128],
                                rhs=xblk[:, kc, :],
                                start=(kc == 0),
                                stop=(kc == DCH - 1),
                            )
                    # RoPE on the 4 q heads (A = first-half dims, B = second)
                    tmp = tmpp.tile([128, SBW], F32, tag="tmp")
                    nc.vector.tensor_tensor(
                        outA[:, sbc], psA[:], cos_s[:, sbc], ALU.mult
                    )
                    nc.vector.tensor_tensor(tmp[:], psB[:], sin_s[:, sbc], ALU.mult)
                    nc.vector.tensor_tensor(
                        outA[:, sbc], outA[:, sbc], tmp[:], ALU.subtract
                    )
                    tmp2 = tmpp.tile([128, SBW], F32, tag="tmp")
                    nc.vector.tensor_tensor(
                        outB[:, sbc], psB[:], cos_s[:, sbc], ALU.mult
                    )
                    nc.vector.tensor_tensor(tmp2[:], psA[:], sin_s[:, sbc], ALU.mult)
                    nc.vector.tensor_tensor(
                        outB[:, sbc], outB[:, sbc], tmp2[:], ALU.add
                    )
                    # evict k|v rows
                    nc.scalar.activation(kvraw[:, sbc], psKV[:], AF.Copy)

                    # k RoPE on this s-block: kswap = [k_hi; k_lo]
                    nc.sync.dma_start(kswap[0:32, sbc], kvraw[32:64, sbc])
                    nc.sync.dma_start(kswap[32:64, sbc], kvraw[0:32, sbc])
                    nc.vector.tensor_tensor(
                        ktdup[0:64, sbc], kvraw[0:64, sbc], cos_s[0:64, sbc], ALU.mult
                    )
                    tmpk = tmpp.tile([64, SBW], F32, tag="tmpk")
                    nc.vector.tensor_tensor(
                        tmpk[:], kswap[:, sbc], sin_s[0:64, sbc], ALU.mult
                    )
                    nc.vector.tensor_tensor(
                        ktdup[0:32, sbc], ktdup[0:32, sbc], tmpk[0:32, :],
                        ALU.subtract,
                    )
                    nc.vector.tensor_tensor(
                        ktdup[32:64, sbc], ktdup[32:64, sbc], tmpk[32:64, :],
                        ALU.add,
                    )
                    nc.sync.dma_start(ktdup[64:128, sbc], ktdup[0:64, sbc])

                    # v: [64, 512] -> 4 key-chunk tiles [128, 64] via PE transpose
                    for c in range(4 * sb, 4 * sb + 4):
                        ptr = pstr.tile([128, 64], F32, tag="ptr")
                        nc.tensor.transpose(
                            ptr[:],
                            kvraw[64:128, c * 128 : (c + 1) * 128],
                            ident[64:128, 64:128],
                        )
                        nc.vector.tensor_copy(v_s[:, c, 0:64], ptr[:])

            # ---------------- P2: attention ------------------------------------
            with (
                tc.tile_pool(name="qsp", bufs=3) as qsp,
                tc.tile_pool(name="ptp", bufs=10) as ptp,
                tc.tile_pool(name="stgp", bufs=4) as stgp,
                tc.tile_pool(name="pssc", bufs=2, space="PSUM") as pssc,
                tc.tile_pool(name="pso_p", bufs=1, space="PSUM") as pso_p,
                tc.tile_pool(name="psbc", bufs=2, space="PSUM") as psbc,
            ):
                for h in range(HQ):
                    hc = slice(32 * h, 32 * h + 32)
                    qs = qsp.tile([128, S], BF16, tag="qs")
                    for sb in range(NSB):
                        sbc = slice(sb * SBW, (sb + 1) * SBW)
                        nc.sync.dma_start(qs[0:32, sbc], outA[hc, sbc])
                        nc.sync.dma_start(qs[32:64, sbc], outB[hc, sbc])
                        nc.sync.dma_start(qs[64:96, sbc], outA[hc, sbc])
                        nc.sync.dma_start(qs[96:128, sbc], outB[hc, sbc])
                    for qb in range(NQB):
                        q0 = qb * QBW
                        nkc = 4 * (qb + 1)
                        pso = pso_p.tile([128, 1024], F32, tag="pso")
                        for pair in range(nkc // 2):
                            cA, cB = 2 * pair, 2 * pair + 1
                            psc = pssc.tile([128, 1024], F32, tag="psc")
                            ptt = ptp.tile([128, 1024], BF16, tag="ptt")
                            for c, half, r0 in ((cA, 0, 0), (cB, 1, 64)):
                                kc0 = c * 128
                                d = max(0, kc0 - q0)
                                nc.tensor.matmul(
                                    psc[:, half * 512 + d : half * 512 + 512],
                                    lhsT=ktdup[r0 : r0 + 64, kc0 : kc0 + 128],
                                    rhs=qs[r0 : r0 + 64, q0 + d : q0 + QBW],
                                    start=True,
                                    stop=True,
                                    tile_position=(r0, 0),
                                )
                            dA = max(0, cA * 128 - q0)
                            dB = max(0, cB * 128 - q0)
                            if dA == 0 and dB == 0:
                                nc.scalar.activation(ptt[:], psc[:], AF.Exp)
                            else:
                                nc.scalar.activation(
                                    ptt[:, dA:512], psc[:, dA:512], AF.Exp
                                )
                                nc.scalar.activation(
                                    ptt[:, 512 + dB : 1024],
                                    psc[:, 512 + dB : 1024],
                                    AF.Exp,
                                )
                            for c, half in ((cA, 0), (cB, 1)):
                                kc0 = c * 128
                                if kc0 + 127 > q0 + 0:
                                    ww = min(512, (kc0 - q0) + 128)
                                    sl = slice(half * 512, half * 512 + ww)
                                    nc.gpsimd.affine_select(
                                        out=ptt[:, sl],
                                        in_=ptt[:, sl],
                                        compare_op=ALU.is_ge,
                                        fill=0.0,
                                        base=q0 - kc0,
                                        channel_multiplier=-1,
                                        pattern=[[1, ww]],
                                    )
                            for c, half in ((cA, 0), (cB, 1)):
                                for r0, b0 in ((0, 0), (64, 512)):
                                    nc.tensor.matmul(
                                        pso[0:65, b0 : b0 + 512],
                                        lhsT=v_s[r0 : r0 + 64, c, :],
                                        rhs=ptt[
                                            r0 : r0 + 64,
                                            half * 512 : half * 512 + 512,
                                        ],
                                        start=(c == 0),
                                        stop=(c == nkc - 1),
                                        tile_position=(r0, 0),
                                    )
                        # evict raw attn out + sums
                        idx = 4 * h + qb
                        ch = h // 2
                        rr = 64 * (h % 2)
                        stg = stgp.tile([128, QBW], BF16, tag="stg")
                        tmpb = stgp.tile([128, QBW], F32, tag="tmpb")
                        nc.vector.tensor_copy(tmpb[0:65, :], pso[0:65, 512:1024])
                        nc.vector.tensor_tensor(
                            stg[0:65, :], pso[0:65, 0:512], tmpb[0:65, :], ALU.add
                        )
                        nc.sync.dma_start(
                            ao[rr : rr + 64, ch, q0 : q0 + QBW], stg[0:64, :]
                        )
                        nc.gpsimd.dma_start(
                            sums_all[idx : idx + 1, :], stg[64:65, :]
                        )

                    # normalize this head: recip of its 4 sums rows, then
                    # broadcast via select-matmul and scale ao in place
                    nc.vector.reciprocal_approx_accurate(
                        rcp_f32[:], sums_all[:], rcp_scr[:]
                    )
                    nc.vector.tensor_copy(rcp_all[:], rcp_f32[:])
                    for qb in range(NQB):
                        idx = 4 * h + qb
                        ch = h // 2
                        rr = 64 * (h % 2)
                        q0 = qb * QBW
                        pbc = psbc.tile([128, QBW], F32, tag="pbc")
                        nc.tensor.matmul(
                            pbc[:],
                            lhsT=sel_s[:, idx * 128 : (idx + 1) * 128],
                            rhs=rcp_all[:],
                            start=True,
                            stop=True,
                        )
                        nc.vector.tensor_tensor(
                            ao[rr : rr + 64, ch, q0 : q0 + QBW],
                            ao[rr : rr + 64, ch, q0 : q0 + QBW],
                            pbc[rr : rr + 64, :],
                            ALU.mult,
                        )


            # ---------------- P3: o_proj ---------------------------------------
            with (
                tc.tile_pool(name="yp", bufs=8) as yp,
                tc.tile_pool(name="psy", bufs=4, space="PSUM") as psy_p,
            ):
                for st in range(16):
                    for ob in range(4):
                        psy = psy_p.tile([128, 512], F32, tag="psy")
                        for ch in range(2):
                            nc.tensor.matmul(
                                psy[:],
                                lhsT=ao[:, ch, st * 128 : (st + 1) * 128],
                                rhs=wo_s[:, ch, ob * 512 : (ob + 1) * 512],
                                start=(ch == 0),
                                stop=(ch == 1),
                            )
                        ysb = yp.tile([128, 512], F32, tag="ysb")
                        if (st * 4 + ob) % 2 == 0:
                            nc.scalar.activation(ysb[:], psy[:], AF.Copy)
                        else:
                            nc.vector.tensor_copy(ysb[:], psy[:])
                        nc.gpsimd.dma_start(
                            y_d[st * 128 : (st + 1) * 128, ob * 512 : (ob + 1) * 512],
                            ysb[:],
                        )
    nc.compile()
    return nc


def _prep_inputs(x, Wq, Wk, Wv, Wo, inv_freq):
    """Host-side sharding + layout prep. Returns in_maps for the 8 cores."""
    x = np.ascontiguousarray(np.asarray(x, dtype=np.float32).reshape(S, D))
    xt = np.ascontiguousarray(x.T)  # [D, S]

    pos = np.arange(S, dtype=np.float64)
    inv = np.asarray(inv_freq, dtype=np.float64)  # [32]
    freqs = pos[None, :] * inv[:, None]  # [32, S]
    cos32 = np.cos(freqs).astype(np.float32)
    sin32 = np.sin(freqs).astype(np.float32)
    cos_tab = np.tile(cos32, (4, 1))  # [128, S]
    sin_tab = np.tile(sin32, (4, 1))
    sel = np.zeros((16, 16 * 128), dtype=np.float32)
    for i in range(16):
        h = i // 4
        rr = 64 * (h % 2)
        sel[i, i * 128 + rr : i * 128 + rr + 64] = 1.0

    in_maps = []
    for i in range(NCORES):
        wq_l = Wq[256 * i : 256 * (i + 1)].astype(np.float32) * 0.125  # [256, D]
        wk_l = Wk[64 * i : 64 * (i + 1)].astype(np.float32)  # [64, D]
        wv_l = Wv[64 * i : 64 * (i + 1)].astype(np.float32)  # [64, D]
        # A-tile: first-half dims of the 4 heads; B-tile: second halves
        wA = np.concatenate(
            [wq_l[64 * h : 64 * h + 32] for h in range(HQ)], axis=0
        )  # [128, D]
        wB = np.concatenate(
            [wq_l[64 * h + 32 : 64 * h + 64] for h in range(HQ)], axis=0
        )
        wkv = np.concatenate([wk_l, wv_l], axis=0)  # [128, D]
        wqkv = np.ascontiguousarray(
            np.concatenate([wA, wB, wkv], axis=0).T
        )  # [D, 384]
        wo_l = Wo[:, 256 * i : 256 * (i + 1)].astype(np.float32)  # [D, 256]
        wo_t = np.ascontiguousarray(wo_l.T.reshape(2, 128, D))  # [2, 128, D]
        in_maps.append(
            {
                "xt": xt.astype(ml_dtypes.bfloat16),
                "wqkv": wqkv.astype(ml_dtypes.bfloat16),
                "wo": wo_t.astype(ml_dtypes.bfloat16),
                "cos": cos_tab,
                "sin": sin_tab,
                "sel": sel.astype(ml_dtypes.bfloat16),
            }
        )
    return in_maps


_NC_CACHE = None


def kernel(x, Wq, Wk, Wv, Wo, inv_freq):
    global _NC_CACHE
    if _NC_CACHE is None:
        _NC_CACHE = _build_nc()
    nc = _NC_CACHE
    in_maps = _prep_inputs(x, Wq, Wk, Wv, Wo, inv_freq)
    trace = bool(int(os.environ.get("BASS_KERNEL_TRACE", "0")))
    res = run_bass_kernel_spmd(nc, in_maps, list(range(NCORES)), trace=trace)
    if trace:
        kernel.last_results = res
    y = np.zeros((S, D), dtype=np.float32)
    for i in range(NCORES):
        y += res.results[i]["y"]
    return y.reshape(1, S, D)


# revision 17
# speedup vs baseline: 1.1215x; 1.0197x over previous
"""Trainium2 Bass kernel for GQA attention (B=1, S=2048, D=2048, H=32, KV=8, HD=64).

Tensor-parallel over heads across 8 NeuronCores: core i holds q-heads
[4i, 4i+4) and kv-head i; each core computes its partial o_proj output and the
host sums the 8 partials (Megatron all-reduce done host-side).

Self-contained: only imports concourse (on sys.path in the container).
"""

import os
import sys

import ml_dtypes
import numpy as np

if "/opt/trn_rl_repo" not in sys.path and not any(
    p.endswith("trn_rl_repo") for p in sys.path
):
    sys.path.insert(0, "/opt/trn_rl_repo")

import concourse.bass as bass
import concourse.mybir as mybir
import concourse.tile as tile
from concourse import bacc
from concourse.bass_utils import run_bass_kernel_spmd
from concourse.masks import make_identity

F32 = mybir.dt.float32
F32R = mybir.dt.float32r
BF16 = mybir.dt.bfloat16


def _r(ap):
    return ap.bitcast(F32R)
AF = mybir.ActivationFunctionType
ALU = mybir.AluOpType

S = 2048
D = 2048
H = 32
KV = 8
HD = 64
NCORES = 8
HQ = H // NCORES  # 4 q heads per core
NKC = S // 128  # 16 key chunks
NQB = 4  # q blocks of 512
QBW = 512
NSB = 4  # s blocks of 512 in projection
SBW = 512
DCH = D // 128  # 16 contraction chunks

def _build_nc():
    nc = bacc.Bacc("TRN2", target_bir_lowering=False, debug=False, num_devices=NCORES)

    xt_d = nc.declare_dram_parameter("xt", [D, S], BF16, isOutput=False)
    wqkv_d = nc.declare_dram_parameter("wqkv", [D, 384], BF16, isOutput=False)
    wo_d = nc.declare_dram_parameter("wo", [2, 128, D], BF16, isOutput=False)
    cos_d = nc.declare_dram_parameter("cos", [128, S], F32, isOutput=False)
    sin_d = nc.declare_dram_parameter("sin", [128, S], F32, isOutput=False)
    sel_d = nc.declare_dram_parameter("sel", [16, 16 * 128], BF16, isOutput=False)
    y_d = nc.declare_dram_parameter("y", [S, D], F32, isOutput=True)

    with tile.TileContext(nc) as tc:
        with (
            tc.tile_pool(name="glob", bufs=1) as glob,
            tc.tile_pool(name="globp", bufs=2, space="PSUM") as globp,
        ):
            ktdup = glob.tile([128, S], BF16, tag="ktdup")
            v_s = glob.tile([128, NKC, 65], BF16, tag="v_s")
            outA = glob.tile([128, S], BF16, tag="outA")
            outB = glob.tile([128, S], BF16, tag="outB")
            ao = glob.tile([128, 2, S], BF16, tag="ao")
            sel_s = glob.tile([16, 16 * 128], BF16, tag="sel_s")
            ident = glob.tile([128, 128], F32, tag="ident")
            wo_s = glob.tile([128, 2, D], BF16, tag="wo_s")
            sums_all = glob.tile([16, QBW], F32, tag="sums_all")
            rcp_all = glob.tile([16, QBW], BF16, tag="rcp_all")
            rcp_f32 = glob.tile([16, QBW], F32, tag="rcp_f32")
            rcp_scr = glob.tile([16, QBW], F32, tag="rcp_scr")

            make_identity(nc, ident[:])
            nc.sync.dma_start(sel_s[:], sel_d[:])
            for ch in range(2):
                nc.sync.dma_start(wo_s[:, ch, :], wo_d[ch])
            nc.vector.memset(v_s[:, :, 64], 1.0)

            # ---------------- P1: projections + RoPE + v transpose -------------
            with (
                tc.tile_pool(name="p1", bufs=1) as p1,
                tc.tile_pool(name="xp", bufs=3) as xp,
                tc.tile_pool(name="tmpp", bufs=4) as tmpp,
                tc.tile_pool(name="ps1", bufs=6, space="PSUM") as ps1,
                tc.tile_pool(name="pstr", bufs=2, space="PSUM") as pstr,
            ):
                wq_s = p1.tile([128, DCH, 384], BF16, tag="wq_s")
                wqkv_r = wqkv_d.rearrange("(ko p) n -> p ko n", p=128)
                for kc in range(DCH):
                    nc.sync.dma_start(wq_s[:, kc, :], wqkv_r[:, kc, :])
                cos_s = p1.tile([128, S], F32, tag="cos_s")
                sin_s = p1.tile([128, S], F32, tag="sin_s")
                nc.sync.dma_start(cos_s[:], cos_d[:])
                nc.sync.dma_start(sin_s[:], sin_d[:])
                kvraw = p1.tile([128, S], F32, tag="kvraw")
                kswap = p1.tile([64, S], F32, tag="kswap")

                xt_r = xt_d.rearrange("(ko p) s -> p ko s", p=128)
                for sb in range(NSB):
                    sbc = slice(sb * SBW, (sb + 1) * SBW)
                    xblk = xp.tile([128, DCH, SBW], BF16, tag="xblk")
                    for kq in range(4):
                        nc.sync.dma_start(
                            xblk[:, 4 * kq : 4 * kq + 4, :],
                            xt_r[:, 4 * kq : 4 * kq + 4, sbc],
                        )
                    psA = ps1.tile([128, SBW], F32, tag="proj")
                    psB = ps1.tile([128, SBW], F32, tag="proj")
                    psKV = ps1.tile([128, SBW], F32, tag="proj")
                    for ps_t, col0 in ((psA, 0), (psB, 128), (psKV, 256)):
                        for kc in range(DCH):
                            nc.tensor.matmul(
                                ps_t[:],
                                lhsT=wq_s[:, kc, col0 : col0 + 128],
                                rhs=xblk[:, kc, :],
                                start=(kc == 0),
                                stop=(kc == DCH - 1),
                            )
                    # RoPE on the 4 q heads (A = first-half dims, B = second)
                    tmp = tmpp.tile([128, SBW], F32, tag="tmp")
                    nc.vector.tensor_tensor(
                        outA[:, sbc], psA[:], cos_s[:, sbc], ALU.mult
                    )
                    nc.vector.tensor_tensor(tmp[:], psB[:], sin_s[:, sbc], ALU.mult)
                    nc.vector.tensor_tensor(
                        outA[:, sbc], outA[:, sbc], tmp[:], ALU.subtract
                    )
                    tmp2 = tmpp.tile([128, SBW], F32, tag="tmp")
                    nc.vector.tensor_tensor(
                        outB[:, sbc], psB[:], cos_s[:, sbc], ALU.mult
                    )
                    nc.vector.tensor_tensor(tmp2[:], psA[:], sin_s[:, sbc], ALU.mult)
                    nc.vector.tensor_tensor(
                        outB[:, sbc], outB[:, sbc], tmp2[:], ALU.add
                    )
                    # evict k|v rows
                    nc.scalar.activation(kvraw[:, sbc], psKV[:], AF.Copy)

                    # k RoPE on this s-block: kswap = [k_hi; k_lo]
                    nc.sync.dma_start(kswap[0:32, sbc], kvraw[32:64, sbc])
                    nc.sync.dma_start(kswap[32:64, sbc], kvraw[0:32, sbc])
                    nc.vector.tensor_tensor(
                        ktdup[0:64, sbc], kvraw[0:64, sbc], cos_s[0:64, sbc], ALU.mult
                    )
                    tmpk = tmpp.tile([64, SBW], F32, tag="tmpk")
                    nc.vector.tensor_tensor(
                        tmpk[:], kswap[:, sbc], sin_s[0:64, sbc], ALU.mult
                    )
                    nc.vector.tensor_tensor(
                        ktdup[0:32, sbc], ktdup[0:32, sbc], tmpk[0:32, :],
                        ALU.subtract,
                    )
                    nc.vector.tensor_tensor(
                        ktdup[32:64, sbc], ktdup[32:64, sbc], tmpk[32:64, :],
                        ALU.add,
                    )
                    nc.sync.dma_start(ktdup[64:128, sbc], ktdup[0:64, sbc])

                    # v: [64, 512] -> 4 key-chunk tiles [128, 64] via PE transpose
                    for c in range(4 * sb, 4 * sb + 4):
                        ptr = pstr.tile([128, 64], F32, tag="ptr")
                        nc.tensor.transpose(
                            ptr[:],
                            kvraw[64:128, c * 128 : (c + 1) * 128],
                            ident[64:128, 64:128],
                        )
                        nc.vector.tensor_copy(v_s[:, c, 0:64], ptr[:])

            # ---------------- P2: attention ------------------------------------
            with (
                tc.tile_pool(name="qsp", bufs=3) as qsp,
                tc.tile_pool(name="ptp", bufs=10) as ptp,
                tc.tile_pool(name="stgp", bufs=4) as stgp,
                tc.tile_pool(name="pssc", bufs=3, space="PSUM") as pssc,
                tc.tile_pool(name="pso_p", bufs=1, space="PSUM") as pso_p,
            ):
                for h in range(HQ):
                    hc = slice(32 * h, 32 * h + 32)
                    qs = qsp.tile([128, S], BF16, tag="qs")
                    for sb in range(NSB):
                        sbc = slice(sb * SBW, (sb + 1) * SBW)
                        nc.sync.dma_start(qs[0:32, sbc], outA[hc, sbc])
                        nc.sync.dma_start(qs[32:64, sbc], outB[hc, sbc])
                        nc.sync.dma_start(qs[64:96, sbc], outA[hc, sbc])
                        nc.sync.dma_start(qs[96:128, sbc], outB[hc, sbc])
                    for qb in range(NQB):
                        q0 = qb * QBW
                        nkc = 4 * (qb + 1)
                        pso = pso_p.tile([128, 1024], F32, tag="pso")
                        for pair in range(nkc // 2):
                            cA, cB = 2 * pair, 2 * pair + 1
                            psc = pssc.tile([128, 1024], F32, tag="psc")
                            ptt = ptp.tile([128, 1024], BF16, tag="ptt")
                            for c, half, r0 in ((cA, 0, 0), (cB, 1, 64)):
                                kc0 = c * 128
                                d = max(0, kc0 - q0)
                                nc.tensor.matmul(
                                    psc[:, half * 512 + d : half * 512 + 512],
                                    lhsT=ktdup[r0 : r0 + 64, kc0 : kc0 + 128],
                                    rhs=qs[r0 : r0 + 64, q0 + d : q0 + QBW],
                                    start=True,
                                    stop=True,
                                    tile_position=(r0, 0),
                                )
                            dA = max(0, cA * 128 - q0)
                            dB = max(0, cB * 128 - q0)
                            if dA == 0 and dB == 0:
                                nc.scalar.activation(ptt[:], psc[:], AF.Exp)
                            else:
                                nc.scalar.activation(
                                    ptt[:, dA:512], psc[:, dA:512], AF.Exp
                                )
                                nc.scalar.activation(
                                    ptt[:, 512 + dB : 1024],
                                    psc[:, 512 + dB : 1024],
                                    AF.Exp,
                                )
                            for c, half in ((cA, 0), (cB, 1)):
                                kc0 = c * 128
                                if kc0 + 127 > q0 + 0:
                                    ww = min(512, (kc0 - q0) + 128)
                                    sl = slice(half * 512, half * 512 + ww)
                                    nc.gpsimd.affine_select(
                                        out=ptt[:, sl],
                                        in_=ptt[:, sl],
                                        compare_op=ALU.is_ge,
                                        fill=0.0,
                                        base=q0 - kc0,
                                        channel_multiplier=-1,
                                        pattern=[[1, ww]],
                                    )
                            for c, half in ((cA, 0), (cB, 1)):
                                for r0, b0 in ((0, 0), (64, 512)):
                                    nc.tensor.matmul(
                                        pso[0:65, b0 : b0 + 512],
                                        lhsT=v_s[r0 : r0 + 64, c, :],
                                        rhs=ptt[
                                            r0 : r0 + 64,
                                            half * 512 : half * 512 + 512,
                                        ],
                                        start=(c == 0),
                                        stop=(c == nkc - 1),
                                        tile_position=(r0, 0),
                                    )
                        # evict raw attn out + sums
                        idx = 4 * h + qb
                        ch = h // 2
                        rr = 64 * (h % 2)
                        stg = stgp.tile([128, QBW], BF16, tag="stg")
                        tmpb = stgp.tile([128, QBW], F32, tag="tmpb")
                        nc.vector.tensor_copy(tmpb[0:65, :], pso[0:65, 512:1024])
                        nc.vector.tensor_tensor(
                            stg[0:65, :], pso[0:65, 0:512], tmpb[0:65, :], ALU.add
                        )
                        nc.sync.dma_start(
                            ao[rr : rr + 64, ch, q0 : q0 + QBW], stg[0:64, :]
                        )
                        nc.gpsimd.dma_start(
                            sums_all[idx : idx + 1, :], stg[64:65, :]
                        )

                    # normalize this head: recip of its 4 sums rows, then
                    # broadcast via select-matmul and scale ao in place
                    nc.vector.reciprocal_approx_accurate(
                        rcp_f32[:], sums_all[:], rcp_scr[:]
                    )
                    nc.vector.tensor_copy(rcp_all[:], rcp_f32[:])
                    for qb in range(NQB):
                        idx = 4 * h + qb
                        ch = h // 2
                        rr = 64 * (h % 2)
                        q0 = qb * QBW
                        pbc = pssc.tile([128, 1024], F32, tag="psc")
                        nc.tensor.matmul(
                            pbc[:, 0:QBW],
                            lhsT=sel_s[:, idx * 128 : (idx + 1) * 128],
                            rhs=rcp_all[:],
                            start=True,
                            stop=True,
                        )
                        nc.vector.tensor_tensor(
                            ao[rr : rr + 64, ch, q0 : q0 + QBW],
                            ao[rr : rr + 64, ch, q0 : q0 + QBW],
                            pbc[rr : rr + 64, 0:QBW],
                            ALU.mult,
                        )


            # ---------------- P3: o_proj ---------------------------------------
            with (
                tc.tile_pool(name="yp", bufs=8) as yp,
                tc.tile_pool(name="psy", bufs=4, space="PSUM") as psy_p,
            ):
                for st in range(16):
                    for ob in range(4):
                        psy = psy_p.tile([128, 512], F32, tag="psy")
                        for ch in range(2):
                            nc.tensor.matmul(
                                psy[:],
                                lhsT=ao[:, ch, st * 128 : (st + 1) * 128],
                                rhs=wo_s[:, ch, ob * 512 : (ob + 1) * 512],
                                start=(ch == 0),
                                stop=(ch == 1),
                            )
                        ysb = yp.tile([128, 512], F32, tag="ysb")
                        if (st * 4 + ob) % 2 == 0:
                            nc.scalar.activation(ysb[:], psy[:], AF.Copy)
                        else:
                            nc.vector.tensor_copy(ysb[:], psy[:])
                        eng = nc.gpsimd if (st * 4 + ob) % 2 == 0 else nc.sync
                        eng.dma_start(
                            y_d[st * 128 : (st + 1) * 128, ob * 512 : (ob + 1) * 512],
                            ysb[:],
                        )
    nc.compile()
    return nc


def _prep_inputs(x, Wq, Wk, Wv, Wo, inv_freq):
    """Host-side sharding + layout prep. Returns in_maps for the 8 cores."""
    x = np.ascontiguousarray(np.asarray(x, dtype=np.float32).reshape(S, D))
    xt = np.ascontiguousarray(x.T)  # [D, S]

    pos = np.arange(S, dtype=np.float64)
    inv = np.asarray(inv_freq, dtype=np.float64)  # [32]
    freqs = pos[None, :] * inv[:, None]  # [32, S]
    cos32 = np.cos(freqs).astype(np.float32)
    sin32 = np.sin(freqs).astype(np.float32)
    cos_tab = np.tile(cos32, (4, 1))  # [128, S]
    sin_tab = np.tile(sin32, (4, 1))
    sel = np.zeros((16, 16 * 128), dtype=np.float32)
    for i in range(16):
        h = i // 4
        rr = 64 * (h % 2)
        sel[i, i * 128 + rr : i * 128 + rr + 64] = 1.0

    in_maps = []
    for i in range(NCORES):
        wq_l = Wq[256 * i : 256 * (i + 1)].astype(np.float32) * 0.125  # [256, D]
        wk_l = Wk[64 * i : 64 * (i + 1)].astype(np.float32)  # [64, D]
        wv_l = Wv[64 * i : 64 * (i + 1)].astype(np.float32)  # [64, D]
        # A-tile: first-half dims of the 4 heads; B-tile: second halves
        wA = np.concatenate(
            [wq_l[64 * h : 64 * h + 32] for h in range(HQ)], axis=0
        )  # [128, D]
        wB = np.concatenate(
            [wq_l[64 * h + 32 : 64 * h + 64] for h in range(HQ)], axis=0
        )
        wkv = np.concatenate([wk_l, wv_l], axis=0)  # [128, D]
        wqkv = np.ascontiguousarray(
            np.concatenate([wA, wB, wkv], axis=0).T
        )  # [D, 384]
        wo_l = Wo[:, 256 * i : 256 * (i + 1)].astype(np.float32)  # [D, 256]
        wo_t = np.ascontiguousarray(wo_l.T.reshape(2, 128, D))  # [2, 128, D]
        in_maps.append(
            {
                "xt": xt.astype(ml_dtypes.bfloat16),
                "wqkv": wqkv.astype(ml_dtypes.bfloat16),
                "wo": wo_t.astype(ml_dtypes.bfloat16),
                "cos": cos_tab,
                "sin": sin_tab,
                "sel": sel.astype(ml_dtypes.bfloat16),
            }
        )
    return in_maps


_NC_CACHE = None


def kernel(x, Wq, Wk, Wv, Wo, inv_freq):
    global _NC_CACHE
    if _NC_CACHE is None:
        _NC_CACHE = _build_nc()
    nc = _NC_CACHE
    in_maps = _prep_inputs(x, Wq, Wk, Wv, Wo, inv_freq)
    trace = bool(int(os.environ.get("BASS_KERNEL_TRACE", "0")))
    res = run_bass_kernel_spmd(nc, in_maps, list(range(NCORES)), trace=trace)
    if trace:
        kernel.last_results = res
    y = np.zeros((S, D), dtype=np.float32)
    for i in range(NCORES):
        y += res.results[i]["y"]
    return y.reshape(1, S, D)


# revision 19
# speedup vs baseline: 1.2057x; 1.0751x over previous
"""Trainium2 Bass kernel for GQA attention (B=1, S=2048, D=2048, H=32, KV=8, HD=64).

Tensor-parallel over heads across 8 NeuronCores: core i holds q-heads
[4i, 4i+4) and kv-head i; each core computes its partial o_proj output and the
host sums the 8 partials (Megatron all-reduce done host-side).

Self-contained: only imports concourse (on sys.path in the container).
"""

import os
import sys

import ml_dtypes
import numpy as np

if "/opt/trn_rl_repo" not in sys.path and not any(
    p.endswith("trn_rl_repo") for p in sys.path
):
    sys.path.insert(0, "/opt/trn_rl_repo")

import concourse.bass as bass
import concourse.mybir as mybir
import concourse.tile as tile
from concourse import bacc
from concourse.bass_utils import run_bass_kernel_spmd
from concourse.masks import make_identity

F32 = mybir.dt.float32
F32R = mybir.dt.float32r
BF16 = mybir.dt.bfloat16


def _r(ap):
    return ap.bitcast(F32R)
AF = mybir.ActivationFunctionType
ALU = mybir.AluOpType

S = 2048
D = 2048
H = 32
KV = 8
HD = 64
NCORES = 8
HQ = H // NCORES  # 4 q heads per core
NKC = S // 128  # 16 key chunks
NQB = 4  # q blocks of 512
QBW = 512
NSB = 4  # s blocks of 512 in projection
SBW = 512
DCH = D // 128  # 16 contraction chunks

def _build_nc():
    nc = bacc.Bacc("TRN2", target_bir_lowering=False, debug=False, num_devices=NCORES)

    xt_d = nc.declare_dram_parameter("xt", [D, S], BF16, isOutput=False)
    wqkv_d = nc.declare_dram_parameter("wqkv", [D, 384], BF16, isOutput=False)
    wo_d = nc.declare_dram_parameter("wo", [2, 128, D], BF16, isOutput=False)
    cos_d = nc.declare_dram_parameter("cos", [128, S], BF16, isOutput=False)
    sin_d = nc.declare_dram_parameter("sin", [128, S], BF16, isOutput=False)
    sel_d = nc.declare_dram_parameter("sel", [16, 16 * 128], BF16, isOutput=False)
    y_d = nc.declare_dram_parameter("y", [S, D], BF16, isOutput=True)

    with tile.TileContext(nc) as tc:
        with (
            tc.tile_pool(name="glob", bufs=1) as glob,
            tc.tile_pool(name="globp", bufs=2, space="PSUM") as globp,
        ):
            ktdup = glob.tile([128, S], BF16, tag="ktdup")
            v_s = glob.tile([128, NKC, 65], BF16, tag="v_s")
            outA = glob.tile([128, S], BF16, tag="outA")
            outB = glob.tile([128, S], BF16, tag="outB")
            ao = glob.tile([128, 2, S], BF16, tag="ao")
            sel_s = glob.tile([16, 16 * 128], BF16, tag="sel_s")
            ident = glob.tile([128, 128], F32, tag="ident")
            wo_s = glob.tile([128, 2, D], BF16, tag="wo_s")
            sums_all = glob.tile([16, QBW], F32, tag="sums_all")
            rcp_all = glob.tile([16, QBW], BF16, tag="rcp_all")
            rcp_f32 = glob.tile([16, QBW], F32, tag="rcp_f32")
            rcp_scr = glob.tile([16, QBW], F32, tag="rcp_scr")

            nc.vector.memset(v_s[:, :, 64], 1.0)

            # ---------------- P1: projections + RoPE + v transpose -------------
            with (
                tc.tile_pool(name="p1", bufs=1) as p1,
                tc.tile_pool(name="xp", bufs=3) as xp,
                tc.tile_pool(name="tmpp", bufs=4) as tmpp,
                tc.tile_pool(name="ps1", bufs=6, space="PSUM") as ps1,
                tc.tile_pool(name="pstr", bufs=2, space="PSUM") as pstr,
            ):
                wq_s = p1.tile([128, DCH, 384], BF16, tag="wq_s")
                wqkv_r = wqkv_d.rearrange("(ko p) n -> p ko n", p=128)
                for kc in range(DCH):
                    nc.sync.dma_start(wq_s[:, kc, :], wqkv_r[:, kc, :])
                cos_s = p1.tile([128, S], BF16, tag="cos_s")
                sin_s = p1.tile([128, S], BF16, tag="sin_s")
                nc.sync.dma_start(cos_s[:], cos_d[:])
                nc.sync.dma_start(sin_s[:], sin_d[:])
                kvraw = p1.tile([128, S], F32, tag="kvraw")
                kswap = p1.tile([64, S], F32, tag="kswap")
                make_identity(nc, ident[:])

                xt_r = xt_d.rearrange("(ko p) s -> p ko s", p=128)
                for sb in range(NSB):
                    sbc = slice(sb * SBW, (sb + 1) * SBW)
                    xblk = xp.tile([128, DCH, SBW], BF16, tag="xblk")
                    for kq in range(4):
                        nc.sync.dma_start(
                            xblk[:, 4 * kq : 4 * kq + 4, :],
                            xt_r[:, 4 * kq : 4 * kq + 4, sbc],
                        )
                    psA = ps1.tile([128, SBW], F32, tag="proj")
                    psB = ps1.tile([128, SBW], F32, tag="proj")
                    psKV = ps1.tile([128, SBW], F32, tag="proj")
                    for ps_t, col0 in ((psA, 0), (psB, 128), (psKV, 256)):
                        for kc in range(DCH):
                            nc.tensor.matmul(
                                ps_t[:],
                                lhsT=wq_s[:, kc, col0 : col0 + 128],
                                rhs=xblk[:, kc, :],
                                start=(kc == 0),
                                stop=(kc == DCH - 1),
                            )
                    # RoPE on the 4 q heads (A = first-half dims, B = second)
                    tmp = tmpp.tile([128, SBW], F32, tag="tmp")
                    nc.vector.tensor_tensor(
                        outA[:, sbc], psA[:], cos_s[:, sbc], ALU.mult
                    )
                    nc.vector.tensor_tensor(tmp[:], psB[:], sin_s[:, sbc], ALU.mult)
                    nc.vector.tensor_tensor(
                        outA[:, sbc], outA[:, sbc], tmp[:], ALU.subtract
                    )
                    tmp2 = tmpp.tile([128, SBW], F32, tag="tmp")
                    nc.vector.tensor_tensor(
                        outB[:, sbc], psB[:], cos_s[:, sbc], ALU.mult
                    )
                    nc.vector.tensor_tensor(tmp2[:], psA[:], sin_s[:, sbc], ALU.mult)
                    nc.vector.tensor_tensor(
                        outB[:, sbc], outB[:, sbc], tmp2[:], ALU.add
                    )
                    # evict k|v rows
                    nc.scalar.activation(kvraw[:, sbc], psKV[:], AF.Copy)

                    # k RoPE on this s-block: kswap = [k_hi; k_lo]
                    nc.sync.dma_start(kswap[0:32, sbc], kvraw[32:64, sbc])
                    nc.sync.dma_start(kswap[32:64, sbc], kvraw[0:32, sbc])
                    nc.vector.tensor_tensor(
                        ktdup[0:64, sbc], kvraw[0:64, sbc], cos_s[0:64, sbc], ALU.mult
                    )
                    tmpk = tmpp.tile([64, SBW], F32, tag="tmpk")
                    nc.vector.tensor_tensor(
                        tmpk[:], kswap[:, sbc], sin_s[0:64, sbc], ALU.mult
                    )
                    nc.vector.tensor_tensor(
                        ktdup[0:32, sbc], ktdup[0:32, sbc], tmpk[0:32, :],
                        ALU.subtract,
                    )
                    nc.vector.tensor_tensor(
                        ktdup[32:64, sbc], ktdup[32:64, sbc], tmpk[32:64, :],
                        ALU.add,
                    )
                    nc.sync.dma_start(ktdup[64:128, sbc], ktdup[0:64, sbc])

                    # v: [64, 512] -> 4 key-chunk tiles [128, 64] via PE transpose
                    for c in range(4 * sb, 4 * sb + 4):
                        ptr = pstr.tile([128, 64], F32, tag="ptr")
                        nc.tensor.transpose(
                            ptr[:],
                            kvraw[64:128, c * 128 : (c + 1) * 128],
                            ident[64:128, 64:128],
                        )
                        nc.vector.tensor_copy(v_s[:, c, 0:64], ptr[:])

            # ---------------- P2: attention ------------------------------------
            with (
                tc.tile_pool(name="qsp", bufs=3) as qsp,
                tc.tile_pool(name="ptp", bufs=10) as ptp,
                tc.tile_pool(name="stgp", bufs=4) as stgp,
                tc.tile_pool(name="pssc", bufs=2, space="PSUM") as pssc,
                tc.tile_pool(name="pso_p", bufs=4, space="PSUM") as pso_p,
            ):
                nc.sync.dma_start(sel_s[:], sel_d[:])
                for ch in range(2):
                    nc.sync.dma_start(wo_s[:, ch, :], wo_d[ch])
                for h in range(HQ):
                    hc = slice(32 * h, 32 * h + 32)
                    qs = qsp.tile([128, S], BF16, tag="qs")
                    for sb in range(NSB):
                        sbc = slice(sb * SBW, (sb + 1) * SBW)
                        nc.sync.dma_start(qs[0:32, sbc], outA[hc, sbc])
                        nc.sync.dma_start(qs[32:64, sbc], outB[hc, sbc])
                        nc.sync.dma_start(qs[64:96, sbc], outA[hc, sbc])
                        nc.sync.dma_start(qs[96:128, sbc], outB[hc, sbc])
                    for qb in range(NQB):
                        q0 = qb * QBW
                        nkc = 4 * (qb + 1)
                        pso = pso_p.tile([128, QBW], F32, tag="pso")
                        for pair in range(nkc // 2):
                            cA, cB = 2 * pair, 2 * pair + 1
                            psc = pssc.tile([128, 1024], F32, tag="psc")
                            ptt = ptp.tile([128, 1024], BF16, tag="ptt")
                            for c, half, r0 in ((cA, 0, 0), (cB, 1, 64)):
                                kc0 = c * 128
                                d = max(0, kc0 - q0)
                                nc.tensor.matmul(
                                    psc[:, half * 512 + d : half * 512 + 512],
                                    lhsT=ktdup[r0 : r0 + 64, kc0 : kc0 + 128],
                                    rhs=qs[r0 : r0 + 64, q0 + d : q0 + QBW],
                                    start=True,
                                    stop=True,
                                    tile_position=(r0, 0),
                                )
                            dA = max(0, cA * 128 - q0)
                            dB = max(0, cB * 128 - q0)
                            if dA == 0 and dB == 0:
                                nc.scalar.activation(ptt[:], psc[:], AF.Exp)
                            else:
                                nc.scalar.activation(
                                    ptt[:, dA:512], psc[:, dA:512], AF.Exp
                                )
                                nc.scalar.activation(
                                    ptt[:, 512 + dB : 1024],
                                    psc[:, 512 + dB : 1024],
                                    AF.Exp,
                                )
                            for c, half in ((cA, 0), (cB, 1)):
                                kc0 = c * 128
                                if kc0 + 127 > q0 + 0:
                                    ww = min(512, (kc0 - q0) + 128)
                                    sl = slice(half * 512, half * 512 + ww)
                                    nc.gpsimd.affine_select(
                                        out=ptt[:, sl],
                                        in_=ptt[:, sl],
                                        compare_op=ALU.is_ge,
                                        fill=0.0,
                                        base=q0 - kc0,
                                        channel_multiplier=-1,
                                        pattern=[[1, ww]],
                                    )
                            for c, half in ((cA, 0), (cB, 1)):
                                nc.tensor.matmul(
                                    pso[0:65, :],
                                    lhsT=v_s[:, c, :],
                                    rhs=ptt[:, half * 512 : half * 512 + 512],
                                    start=(c == 0),
                                    stop=(c == nkc - 1),
                                )
                        # evict raw attn out + sums
                        idx = 4 * h + qb
                        ch = h // 2
                        rr = 64 * (h % 2)
                        stg = stgp.tile([128, QBW], BF16, tag="stg")
                        nc.scalar.activation(stg[0:65, :], pso[0:65, :], AF.Copy)
                        nc.sync.dma_start(
                            ao[rr : rr + 64, ch, q0 : q0 + QBW], stg[0:64, :]
                        )
                        nc.gpsimd.dma_start(
                            sums_all[idx : idx + 1, :], stg[64:65, :]
                        )

                    # normalize this head: recip of its 4 sums rows, then
                    # broadcast via select-matmul and scale ao in place
                    nc.vector.reciprocal_approx_accurate(
                        rcp_f32[:], sums_all[:], rcp_scr[:]
                    )
                    nc.vector.tensor_copy(rcp_all[:], rcp_f32[:])
                    for qb in range(NQB):
                        idx = 4 * h + qb
                        ch = h // 2
                        rr = 64 * (h % 2)
                        q0 = qb * QBW
                        pbc = pssc.tile([128, 1024], F32, tag="psc")
                        nc.tensor.matmul(
                            pbc[:, 0:QBW],
                            lhsT=sel_s[:, idx * 128 : (idx + 1) * 128],
                            rhs=rcp_all[:],
                            start=True,
                            stop=True,
                        )
                        nc.vector.tensor_tensor(
                            ao[rr : rr + 64, ch, q0 : q0 + QBW],
                            ao[rr : rr + 64, ch, q0 : q0 + QBW],
                            pbc[rr : rr + 64, 0:QBW],
                            ALU.mult,
                        )


            # ---------------- P3: o_proj ---------------------------------------
            with (
                tc.tile_pool(name="yp", bufs=8) as yp,
                tc.tile_pool(name="psy", bufs=4, space="PSUM") as psy_p,
            ):
                for st in range(16):
                    for ob in range(4):
                        psy = psy_p.tile([128, 512], F32, tag="psy")
                        for ch in range(2):
                            nc.tensor.matmul(
                                psy[:],
                                lhsT=ao[:, ch, st * 128 : (st + 1) * 128],
                                rhs=wo_s[:, ch, ob * 512 : (ob + 1) * 512],
                                start=(ch == 0),
                                stop=(ch == 1),
                            )
                        ysb = yp.tile([128, 512], BF16, tag="ysb")
                        if (st * 4 + ob) % 2 == 0:
                            nc.scalar.activation(ysb[:], psy[:], AF.Copy)
                        else:
                            nc.vector.tensor_copy(ysb[:], psy[:])
                        eng = nc.gpsimd if (st * 4 + ob) % 2 == 0 else nc.sync
                        eng.dma_start(
                            y_d[st * 128 : (st + 1) * 128, ob * 512 : (ob + 1) * 512],
                            ysb[:],
                        )
    nc.compile()
    return nc


def _prep_inputs(x, Wq, Wk, Wv, Wo, inv_freq):
    """Host-side sharding + layout prep. Returns in_maps for the 8 cores."""
    x = np.ascontiguousarray(np.asarray(x, dtype=np.float32).reshape(S, D))
    xt = np.ascontiguousarray(x.T)  # [D, S]

    pos = np.arange(S, dtype=np.float64)
    inv = np.asarray(inv_freq, dtype=np.float64)  # [32]
    freqs = pos[None, :] * inv[:, None]  # [32, S]
    cos32 = np.cos(freqs).astype(np.float32)
    sin32 = np.sin(freqs).astype(np.float32)
    cos_tab = np.tile(cos32, (4, 1))  # [128, S]
    sin_tab = np.tile(sin32, (4, 1))
    sel = np.zeros((16, 16 * 128), dtype=np.float32)
    for i in range(16):
        h = i // 4
        rr = 64 * (h % 2)
        sel[i, i * 128 + rr : i * 128 + rr + 64] = 1.0

    in_maps = []
    for i in range(NCORES):
        wq_l = Wq[256 * i : 256 * (i + 1)].astype(np.float32) * 0.125  # [256, D]
        wk_l = Wk[64 * i : 64 * (i + 1)].astype(np.float32)  # [64, D]
        wv_l = Wv[64 * i : 64 * (i + 1)].astype(np.float32)  # [64, D]
        # A-tile: first-half dims of the 4 heads; B-tile: second halves
        wA = np.concatenate(
            [wq_l[64 * h : 64 * h + 32] for h in range(HQ)], axis=0
        )  # [128, D]
        wB = np.concatenate(
            [wq_l[64 * h + 32 : 64 * h + 64] for h in range(HQ)], axis=0
        )
        wkv = np.concatenate([wk_l, wv_l], axis=0)  # [128, D]
        wqkv = np.ascontiguousarray(
            np.concatenate([wA, wB, wkv], axis=0).T
        )  # [D, 384]
        wo_l = Wo[:, 256 * i : 256 * (i + 1)].astype(np.float32)  # [D, 256]
        wo_t = np.ascontiguousarray(wo_l.T.reshape(2, 128, D))  # [2, 128, D]
        in_maps.append(
            {
                "xt": xt.astype(ml_dtypes.bfloat16),
                "wqkv": wqkv.astype(ml_dtypes.bfloat16),
                "wo": wo_t.astype(ml_dtypes.bfloat16),
                "cos": cos_tab.astype(ml_dtypes.bfloat16),
                "sin": sin_tab.astype(ml_dtypes.bfloat16),
                "sel": sel.astype(ml_dtypes.bfloat16),
            }
        )
    return in_maps


_NC_CACHE = None


def kernel(x, Wq, Wk, Wv, Wo, inv_freq):
    global _NC_CACHE
    if _NC_CACHE is None:
        _NC_CACHE = _build_nc()
    nc = _NC_CACHE
    in_maps = _prep_inputs(x, Wq, Wk, Wv, Wo, inv_freq)
    trace = bool(int(os.environ.get("BASS_KERNEL_TRACE", "0")))
    res = run_bass_kernel_spmd(nc, in_maps, list(range(NCORES)), trace=trace)
    if trace:
        kernel.last_results = res
    y = np.zeros((S, D), dtype=np.float32)
    for i in range(NCORES):
        y += res.results[i]["y"].astype(np.float32)
    return y.reshape(1, S, D)
